# revision 29
# baseline (speedup 1.0000x reference)
"""Trainium2 Bass kernel for nn_NoFoDifformer_CL (dense spectral transformer + CL loss).

Row-parallel over N=8192 across 8 cores (R=1024 rows/core). All weights
replicated. Collectives: AG(new_e), AR(u^T h), AR(k^T v), AG(zfn), AR(stats).

Layout conventions (per core):
  FM (feature-major): [C, R]  — activations for feature-contraction GEMMs
  NM (node-major):    [R, C]  — activations for node-contraction GEMMs
A GEMM out = kxm.T @ kxn with a feature-major input can emit either layout
(input as kxn -> FM out; input as kxm -> NM out), so the network needs no
on-device transposes.

Key identity exploited: hf = u @ (new_e * (u.T @ h)) == henc (the reference
computes the same spectral propagation twice on the same input h).
"""
import numpy as np
import ml_dtypes
from contextlib import ExitStack

import concourse.bass as bass
import concourse.tile as tile
from concourse import bacc, mybir
from concourse.bass_utils import run_bass_kernel_spmd
from concourse.kernels.tile_matmul import (
    composable_matmul_tile_kernel,
    dma_from_dram_kxm,
    dma_from_dram_kxn,
    dma_to_dram_mxn,
    accumulate_dma_from_dram_mxn,
    ShapeInfo,
)

F32 = mybir.dt.float32
F32R = mybir.dt.float32r
BF16 = mybir.dt.bfloat16
AF = mybir.ActivationFunctionType
ALU = mybir.AluOpType
ts, ds = bass.ts, bass.ds

NCORES = 8
N = 8192
R = N // NCORES          # 1024 rows per core
NFEAT = 512
HD = 512
C = 256                  # transformer width (NC)
H = 1024                 # NH * NC
PD = 128                 # projection dim (NC // 2)
KPOW = 10
NF = 16
TEMP = 0.5
EPS = 1e-5
TWO_PI = float(2 * np.pi)
HALF_PI = float(np.pi / 2)
P = 128

STATS_LEN = N + 2        # [colsum(8192) | sum_i(rowlse-diag) | sum_i diag]


def r32(ap):
    return ap.bitcast(F32R)


# ----------------------------------------------------------------------------
# GEMM wrapper: out = kxm.T @ kxn with custom psum->sbuf reducer + consumer
# ----------------------------------------------------------------------------

def gemm(tc, kxm_ap, kxn_ap, mxn_ap, *, reducer, consumer=None,
         kxm_bufs=3, kxn_bufs=3, kxm_producer=None, kxm_shape=None,
         kxn_producer=None, kxn_shape=None,
         cache_tiles=True, out_dtype=F32):
    nc = tc.nc
    with ExitStack() as ctx:
        if kxm_producer is None:
            kxm_pool = ctx.enter_context(tc.tile_pool(name="kxm_pool", bufs=kxm_bufs))
            kxm_producer, kxm_shape = dma_from_dram_kxm(kxm_pool, kxm_ap)
        if kxn_producer is None:
            kxn_pool = ctx.enter_context(tc.tile_pool(name="kxn_pool", bufs=kxn_bufs))
            kxn_producer, kxn_shape = dma_from_dram_kxn(kxn_pool, kxn_ap)
        if consumer is None:
            consumer = dma_to_dram_mxn(mxn_ap)
        elif isinstance(consumer, tuple):
            # (accumulate DRAM aps...) -> add then DMA out
            acc_aps = consumer
            consumer = dma_to_dram_mxn(mxn_ap)
            acc_pool = ctx.enter_context(tc.tile_pool(name="acc_pool", bufs=3))
            for a in acc_aps:
                consumer = accumulate_dma_from_dram_mxn(consumer, acc_pool, a)
        composable_matmul_tile_kernel(
            tc=tc,
            kxm_shape=kxm_shape,
            kxn_shape=kxn_shape,
            output_type=out_dtype,
            kxm_producer=kxm_producer,
            kxn_producer=kxn_producer,
            mxn_subtile_reducer=reducer,
            mxn_consumer=consumer,
            cache_tiles=cache_tiles,
        )


def red_act(func, bias_sb=None, scale=1.0):
    """ACT psum->sbuf eviction: out = func(psum*scale + bias[m128])."""
    def r(nc, psum, sbuf, md):
        kw = {}
        if bias_sb is not None:
            m128 = md.m_tile_idx * md.m_subtiles + md.m_subtile_idx
            kw["bias"] = bias_sb[:, m128:m128 + 1]
        nc.scalar.activation(sbuf[:], psum[:], func, scale=scale, **kw)
    return r


def red_copy():
    # DVE copy: ACT is the loaded engine (exp/gelu/bias evictions)
    def r(nc, psum, sbuf, md):
        nc.vector.tensor_copy(sbuf[:], psum[:])
    return r


def red_addrep(rep_sb):
    """DVE eviction adding a [128, W] replicated (per-free-position) bias."""
    def r(nc, psum, sbuf, md):
        n0 = md.n_tile_idx * md.n_tile + md.n_subtile_idx * md.n_subtile
        w = min(md.n_subtile, md.n_slice_size)
        nc.vector.tensor_tensor(sbuf[:], psum[:], rep_sb[:, n0:n0 + w], op=ALU.add)
    return r


# ----------------------------------------------------------------------------
# Program builder
# ----------------------------------------------------------------------------

def build_program():
    nc = bacc.Bacc("TRN2", target_bir_lowering=False, debug=False,
                   num_devices=NCORES)

    def din(name, shape, dtype=F32):
        return nc.dram_tensor(name, shape, dtype, kind="ExternalInput")

    def dout(name, shape):
        return nc.dram_tensor(name, shape, F32, kind="ExternalOutput")

    def dint(name, shape, shared=False, dtype=F32):
        if shared:
            return nc.dram_tensor(name, shape, dtype, addr_space="Shared")
        return nc.dram_tensor(name, shape, dtype)

    # ---- external inputs (per-core shards + replicated weights) ----
    xT = din("xT", [NFEAT, R])
    u_r = din("u_r", [R, N], BF16)
    uT_r = din("uT_r", [N, R], BF16)
    ep = din("ep", [P, R // P])          # e in [p, t] layout (row = t*128+p)
    w_fe1 = din("w_fe1", [NFEAT, HD])
    w_fe2 = din("w_fe2", [HD, C])
    w_q = din("w_q", [C, H])
    w_k = din("w_k", [C, H])
    w_v = din("w_v", [C, H])
    w_o = din("w_o", [H, C])
    w_f1 = din("w_f1", [C, C])
    w_f2 = din("w_f2", [C, C])
    w_pt1 = din("w_pt1", [C, C])
    w_pt2 = din("w_pt2", [C, PD])
    w_pf1 = din("w_pf1", [C, C])
    w_pf2 = din("w_pf2", [C, PD])
    w_mo = din("w_mo", [2 * C, C])
    # biases: [T, 128, 1] = per-partition (FM outputs); _rep = [128, W] (NM)
    b_fe1 = din("b_fe1", [HD // P, P, 1])
    b_fe2 = din("b_fe2", [C // P, P, 1])
    b_fe2_rep = din("b_fe2_rep", [P, C])
    b_q = din("b_q", [H // P, P, 1])
    b_k_rep = din("b_k_rep", [P, H])
    b_v_rep = din("b_v_rep", [P, H])
    b_o = din("b_o", [C // P, P, 1])
    b_f1 = din("b_f1", [C // P, P, 1])
    b_f2 = din("b_f2", [C // P, P, 1])
    b_pt1 = din("b_pt1", [C // P, P, 1])
    b_pt2 = din("b_pt2", [PD // P, P, 1])
    b_pf1 = din("b_pf1", [C // P, P, 1])
    b_pf2 = din("b_pf2", [PD // P, P, 1])
    b_mo = din("b_mo", [C // P, P, 1])
    l1g = din("l1g", [C // P, P, 1])
    l1b = din("l1b", [C // P, P, 1])
    l2g = din("l2g", [C // P, P, 1])
    l2b = din("l2b", [C // P, P, 1])
    # eig-encoder consts replicated over partitions:
    # [freqs(16) | wsin(160) | wcos(160) | c0(1)]
    ENC_W = NF + 2 * KPOW * NF + 1
    enc_c = din("enc_c", [P, ENC_W])
    ones_d = din("ones_d", [P, P])

    # ---- outputs ----
    logits_t = dout("logits_t", [C, R])
    cl_out = dout("cl_out", [1, 1])

    # ---- internal DRAM ----
    h1T = dint("h1T", [HD, R])
    hT = dint("hT", [C, R])
    h_nm = dint("h_nm", [R, C], dtype=BF16)
    mhT = dint("mhT", [C, R])
    Ppart = dint("Ppart", [N, C], dtype=BF16)
    Pfull = dint("Pfull", [N, C], shared=True, dtype=BF16)
    new_e_r = dint("new_e_r", [R])
    new_e_all = dint("new_e_all", [N], shared=True)
    kk_nm = dint("kk_nm", [R, H])
    Gwo_part = dint("Gwo_part", [H, C])
    Gwo_full = dint("Gwo_full", [H, C], shared=True)
    midT = dint("midT", [C, R])
    fT = dint("fT", [C, R])
    g1T = dint("g1T", [C, R])
    hthfT = dint("hthfT", [2 * C, R])      # rows 0:256 = htT, 256:512 = hencT
    z1tT = dint("z1tT", [C, R])
    ztT = dint("ztT", [PD, R])
    z1fT = dint("z1fT", [C, R])
    zfT = dint("zfT", [PD, R])
    zfnT = dint("zfnT", [PD, R])
    zfn_all = dint("zfn_all", [NCORES * PD, R], shared=True)
    stats_in = dint("stats_in", [STATS_LEN])
    stats_out = dint("stats_out", [STATS_LEN], shared=True)

    RG = [list(range(NCORES))]

    with tile.TileContext(nc) as tc:
        with ExitStack() as ctx:
            const = ctx.enter_context(tc.tile_pool(name="const", bufs=1))

            ones_c = const.tile([P, 1], F32R)
            nc.sync.dma_start(ones_c[:], r32(ones_d.ap()[:, 0:1]))
            ones_row = const.tile([1, P], F32R)
            nc.sync.dma_start(ones_row[:], r32(ones_d.ap()[0:1, :]))
            ones_f = const.tile([P, 1], F32)
            nc.sync.dma_start(ones_f[:], ones_d.ap()[:, 0:1])

            def load_bias(b):
                w = b.shape[0]
                t = const.tile([P, w], F32, name=f"sb_{b.name}")
                nc.sync.dma_start(t[:], b.ap().rearrange("t p o -> p (t o)"))
                return t

            b_fe1_sb = load_bias(b_fe1)
            b_fe2_sb = load_bias(b_fe2)
            b_q_sb = load_bias(b_q)
            b_o_sb = load_bias(b_o)
            b_f1_sb = load_bias(b_f1)
            b_f2_sb = load_bias(b_f2)
            b_pt1_sb = load_bias(b_pt1)
            b_pt2_sb = load_bias(b_pt2)
            b_pf1_sb = load_bias(b_pf1)
            b_pf2_sb = load_bias(b_pf2)
            b_mo_sb = load_bias(b_mo)
            l1g_sb = load_bias(l1g)
            l1b_sb = load_bias(l1b)
            l2g_sb = load_bias(l2g)
            l2b_sb = load_bias(l2b)
            b_fe2_rep_sb = const.tile([P, C], F32)
            nc.sync.dma_start(b_fe2_rep_sb[:], b_fe2_rep.ap())
            b_k_rep_sb = const.tile([P, H], F32)
            nc.sync.dma_start(b_k_rep_sb[:], b_k_rep.ap())
            b_v_rep_sb = const.tile([P, H], F32)
            nc.sync.dma_start(b_v_rep_sb[:], b_v_rep.ap())


            # ================= feat encoder =================
            # h1T = relu(W1.T x.T + b1)          [512, R] FM
            gemm(tc, r32(w_fe1.ap()), r32(xT.ap()), h1T.ap(),
                 reducer=red_act(AF.Relu, b_fe1_sb))
            # hT = W2.T h1 + b2                  [256, R] FM
            gemm(tc, r32(w_fe2.ap()), r32(h1T.ap()), hT.ap(),
                 reducer=red_act(AF.Identity, b_fe2_sb))
            # h_nm = h1 @ W2 + b2                [R, 256] NM, bf16
            gemm(tc, r32(h1T.ap()), r32(w_fe2.ap()), h_nm.ap(),
                 reducer=red_addrep(b_fe2_rep_sb), out_dtype=BF16)

            # ================= eig encoder -> new_e_r =================
            with ExitStack() as ectx:
                epool = ectx.enter_context(tc.tile_pool(name="enc", bufs=1))
                T8 = R // P  # 8
                encc = epool.tile([P, ENC_W], F32)
                nc.sync.dma_start(encc[:], enc_c.ap())
                freqs_ap = encc[:, 0:NF]
                wsin_ap = encc[:, NF:NF + KPOW * NF]
                wcos_ap = encc[:, NF + KPOW * NF:NF + 2 * KPOW * NF]
                c0_ap = encc[:, NF + 2 * KPOW * NF:ENC_W]

                e_sb = epool.tile([P, T8], F32)
                nc.sync.dma_start(e_sb[:], ep.ap())
                # powers e^(k+1), layout [P, t, k] (t-major)
                pw = epool.tile([P, T8, KPOW], F32)
                nc.vector.tensor_copy(pw[:, :, 0], e_sb[:])
                for k in range(1, KPOW):
                    nc.vector.tensor_tensor(pw[:, :, k], pw[:, :, k - 1], e_sb[:],
                                            op=ALU.mult)
                pw4 = pw[:].rearrange("p t (k o) -> p t k o", k=KPOW)\
                    .broadcast_to([P, T8, KPOW, NF])
                fr4 = freqs_ap.rearrange("p (a b f) -> p a b f", a=1, b=1)\
                    .broadcast_to([P, T8, KPOW, NF])
                NPH = T8 * KPOW * NF
                ph = epool.tile([P, T8, KPOW, NF], F32)
                nc.vector.tensor_tensor(ph[:], pw4, fr4, op=ALU.mult)
                ph2 = ph[:].rearrange("p t k f -> p (t k f)")

                def redsin(src2d, bias):
                    """sin(src + bias), arg range-reduced to [-pi, pi].

                    y = (src+bias)/2pi is in [0, ~4); round(y) is computed
                    exactly as sum of step functions (no dtype-cast rounding
                    ambiguity between sim and HW).
                    """
                    y = epool.tile([P, NPH], F32, name="rr_y")
                    nc.vector.tensor_scalar(y[:], src2d, 1.0 / TWO_PI,
                                            bias / TWO_PI, op0=ALU.mult,
                                            op1=ALU.add)
                    kf = epool.tile([P, NPH], F32, name="rr_kf")
                    g = epool.tile([P, NPH], F32, name="rr_g")
                    for j in range(4):
                        dst = kf if j == 0 else g
                        nc.vector.tensor_scalar(dst[:], y[:], j + 0.5, None,
                                                op0=ALU.is_ge)
                        if j > 0:
                            nc.vector.tensor_tensor(kf[:], kf[:], g[:],
                                                    op=ALU.add)
                    nc.vector.tensor_tensor(y[:], y[:], kf[:], op=ALU.subtract)
                    nc.vector.tensor_scalar(y[:], y[:], TWO_PI, None, op0=ALU.mult)
                    s = epool.tile([P, NPH], F32, name=f"rr_s{bias!r}")
                    nc.scalar.activation(s[:], y[:], AF.Sin)
                    return s

                sv = redsin(ph2, 0.0)
                cv = redsin(ph2, HALF_PI)
                ws4 = wsin_ap.rearrange("p (a k f) -> p a k f", a=1, k=KPOW)\
                    .broadcast_to([P, T8, KPOW, NF])
                wc4 = wcos_ap.rearrange("p (a k f) -> p a k f", a=1, k=KPOW)\
                    .broadcast_to([P, T8, KPOW, NF])
                sv4 = sv[:].rearrange("p (t k f) -> p t k f", t=T8, k=KPOW)
                cv4 = cv[:].rearrange("p (t k f) -> p t k f", t=T8, k=KPOW)
                nc.vector.tensor_tensor(sv4, sv4, ws4, op=ALU.mult)
                nc.vector.tensor_tensor(cv4, cv4, wc4, op=ALU.mult)
                nc.vector.tensor_tensor(sv[:], sv[:], cv[:], op=ALU.add)
                nev = epool.tile([P, T8], F32)
                nc.vector.reduce_sum(
                    nev[:], sv[:].rearrange("p (t kf) -> p t kf", t=T8),
                    axis=mybir.AxisListType.X)
                nc.vector.tensor_scalar(nev[:], nev[:], c0_ap, None, op0=ALU.add)
                nc.sync.dma_start(
                    new_e_r.ap().rearrange("(t p) -> p t", p=P), nev[:])

            nc.gpsimd.collective_compute(
                "AllGather", ALU.bypass, ins=[new_e_r.ap().opt()],
                outs=[new_e_all.ap().opt()], replica_groups=RG)

            # ================= spectral GEMM1: Ppart = u_r.T @ h_r =========
            # Split over the eig axis so the AllReduce of the first half
            # overlaps GEMM1's second half, and GEMM2's first K-tiles can
            # start while the second AR is in flight.
            NSP = 2
            EC = N // NSP
            for c in range(NSP):
                gemm(tc, u_r.ap()[:, c * EC:(c + 1) * EC],
                     h_nm.ap(), Ppart.ap()[c * EC:(c + 1) * EC, :],
                     reducer=red_copy(), kxm_bufs=6, kxn_bufs=4,
                     out_dtype=BF16)
                nc.gpsimd.collective_compute(
                    "AllReduce", ALU.add,
                    ins=[Ppart.ap()[c * EC:(c + 1) * EC, :].opt()],
                    outs=[Pfull.ap()[c * EC:(c + 1) * EC, :].opt()],
                    replica_groups=RG)

            # ================= LN1: mhT = layernorm(hT) =================
            def layernorm_fm(src_dram, dst_dram, g_sb, b_sb, tag):
                """Feature-major layernorm over C=256 features (2 P-tiles).

                Matmul free dim <= 512 (fp32 PSUM bank), so all R-wide
                matmuls run in 512-wide halves.
                """
                with ExitStack() as lctx:
                    lp = lctx.enter_context(
                        tc.tile_pool(name=f"ln_{tag}", bufs=1))
                    lps = lctx.enter_context(
                        tc.tile_pool(name=f"lnp_{tag}", bufs=1, space="PSUM"))
                    NT = C // P   # 2 feature tiles
                    NH2 = R // 512  # 2 column halves
                    hx = []
                    for i in range(NT):
                        t = lp.tile([P, R], F32R, name=f"lnx{i}")
                        nc.sync.dma_start(
                            t[:], r32(src_dram.ap()[i * P:(i + 1) * P, :]))
                        hx.append(t)
                    ps_s = lps.tile([1, R], F32, name="ps_s")
                    ps_q = lps.tile([1, R], F32, name="ps_q")
                    for h in range(NH2):
                        sl = slice(h * 512, (h + 1) * 512)
                        for i in range(NT):
                            nc.tensor.matmul(ps_s[:, sl], ones_c[:],
                                             hx[i][:, sl],
                                             start=(i == 0), stop=(i == NT - 1))
                    sq = []
                    for i in range(NT):
                        t = lp.tile([P, R], F32R, name=f"lnsq{i}")
                        nc.scalar.activation(
                            t[:], hx[i][:].bitcast(F32), AF.Square)
                        sq.append(t)
                    for h in range(NH2):
                        sl = slice(h * 512, (h + 1) * 512)
                        for i in range(NT):
                            nc.tensor.matmul(ps_q[:, sl], ones_c[:],
                                             sq[i][:, sl],
                                             start=(i == 0), stop=(i == NT - 1))
                    mrow = lp.tile([1, R], F32R, name="mrow")
                    nc.scalar.activation(mrow[:], ps_s[:], AF.Copy,
                                         scale=1.0 / C)
                    qrow = lp.tile([1, R], F32, name="qrow")
                    nc.scalar.activation(qrow[:], ps_q[:], AF.Copy,
                                         scale=1.0 / C)
                    mf = mrow[:].bitcast(F32)
                    msq = lp.tile([1, R], F32, name="msq")
                    nc.vector.tensor_tensor(msq[:], mf, mf, op=ALU.mult)
                    # msq - EPS so that qrow - msq = var + EPS (Sqrt bias
                    # can't take arbitrary float consts)
                    nc.vector.tensor_scalar(msq[:], msq[:], EPS, None,
                                            op0=ALU.subtract)
                    nc.vector.tensor_tensor(qrow[:], qrow[:], msq[:],
                                            op=ALU.subtract)
                    srow = lp.tile([1, R], F32, name="srow")
                    nc.scalar.activation(srow[:], qrow[:], AF.Sqrt)
                    rsrow = lp.tile([1, R], F32, name="rsrow")
                    nc.vector.reciprocal(rsrow[:], srow[:])
                    rsr = lp.tile([1, R], F32R, name="rsr")
                    nc.scalar.copy(rsr[:], rsrow[:])
                    ps_mb = lps.tile([P, R], F32, name="ps_mb")
                    ps_rb = lps.tile([P, R], F32, name="ps_rb")
                    for h in range(NH2):
                        sl = slice(h * 512, (h + 1) * 512)
                        nc.tensor.matmul(ps_mb[:, sl], ones_row[:],
                                         mrow[:, sl], start=True, stop=True)
                        nc.tensor.matmul(ps_rb[:, sl], ones_row[:],
                                         rsr[:, sl], start=True, stop=True)
                    for i in range(NT):
                        t1 = lp.tile([P, R], F32, name=f"lnt{i}")
                        nc.vector.tensor_tensor(
                            t1[:], hx[i][:].bitcast(F32), ps_mb[:],
                            op=ALU.subtract)
                        nc.vector.tensor_tensor(t1[:], t1[:], ps_rb[:],
                                                op=ALU.mult)
                        nc.vector.tensor_scalar(
                            t1[:], t1[:], g_sb[:, i:i + 1], b_sb[:, i:i + 1],
                            op0=ALU.mult, op1=ALU.add)
                        nc.sync.dma_start(
                            dst_dram.ap()[i * P:(i + 1) * P, :], t1[:])

            layernorm_fm(hT, mhT, l1g_sb, l1b_sb, "ln1")

            # ================= attention projections =================
            # qT = Wq.T mh + bq kept in SBUF     [1024, R] FM
            q_pool = ctx.enter_context(tc.tile_pool(name="q_sbuf", bufs=1))
            q_sb = q_pool.tile([P, H // P, R], F32R)

            def red_q(nc_, psum, sbuf, md):
                po = md.m_tile_idx * md.m_subtiles + md.m_subtile_idx
                n0 = md.n_tile_idx * md.n_tile + md.n_subtile_idx * md.n_subtile
                w = min(md.n_subtile, md.n_slice_size)
                nc_.scalar.activation(q_sb[:, po, n0:n0 + w], psum[:],
                                      AF.Identity, bias=b_q_sb[:, po:po + 1])

            def q_no_consume(nc_, mxn_tile, md):
                pass

            gemm(tc, r32(w_q.ap()), r32(mhT.ap()), None,
                 reducer=red_q, consumer=q_no_consume)

            def q_producer(nc_, md):
                return q_sb[:, ts(md.k_tile_idx, md.k_subtiles),
                            ts(md.n_tile_idx, md.n_tile)]

            q_shape = ShapeInfo(pdims=((P, H // P),), fdims=(R,))
            # kk = mh @ Wk + bk                  [R, 1024] NM
            gemm(tc, r32(mhT.ap()), r32(w_k.ap()), kk_nm.ap(),
                 reducer=red_addrep(b_k_rep_sb))
            # vv = mh @ Wv + bv kept entirely in SBUF (saves an 8MB round
            # trip: the G' GEMM's kxm producer returns slices of vv_sb)
            with ExitStack() as gtctx:
                gt_pool = gtctx.enter_context(
                    tc.tile_pool(name="gt_sbuf", bufs=1))
                gt_sb = gt_pool.tile([P, H // P, H], F32R)
                vv_sb = gt_pool.tile([P, R // P, H], F32R)

                def red_vv(nc_, psum, sbuf, md):
                    po = md.m_tile_idx * md.m_subtiles + md.m_subtile_idx
                    n0 = md.n_tile_idx * md.n_tile + md.n_subtile_idx * md.n_subtile
                    w = min(md.n_subtile, md.n_slice_size)
                    nc_.vector.tensor_tensor(
                        vv_sb[:, po, n0:n0 + w], psum[:],
                        b_v_rep_sb[:, n0:n0 + w], op=ALU.add)

                def no_consume(nc_, mxn_tile, md):
                    pass

                gemm(tc, r32(mhT.ap()), r32(w_v.ap()), None,
                     reducer=red_vv, consumer=no_consume)

                def red_to_gt(nc_, psum, sbuf, md):
                    po = md.m_tile_idx * md.m_subtiles + md.m_subtile_idx
                    n0 = md.n_tile_idx * md.n_tile + md.n_subtile_idx * md.n_subtile
                    w = min(md.n_subtile, md.n_slice_size)
                    nc_.vector.tensor_copy(gt_sb[:, po, n0:n0 + w], psum[:])

                def vv_producer(nc_, md):
                    return vv_sb[:, ts(md.k_tile_idx, md.k_subtiles),
                                 ts(md.m_tile_idx, md.m_tile)]

                vv_shape = ShapeInfo(pdims=((P, R // P),), fdims=(H,))
                # GT = vv.T @ kk, also SBUF-resident
                gemm(tc, None, r32(kk_nm.ap()), None,
                     reducer=red_to_gt, consumer=no_consume,
                     kxm_producer=vv_producer, kxm_shape=vv_shape,
                     kxn_bufs=5)

                def gt_producer(nc_, md):
                    return gt_sb[:, ts(md.k_tile_idx, md.k_subtiles),
                                 ts(md.m_tile_idx, md.m_tile)]

                gt_shape = ShapeInfo(pdims=((P, H // P),), fdims=(H,))
                # Gwo_part = GT.T @ wo  [1024, 256]
                gemm(tc, None, r32(w_o.ap()), Gwo_part.ap(),
                     reducer=red_copy(), kxm_producer=gt_producer,
                     kxm_shape=gt_shape, kxn_bufs=3)
            nc.gpsimd.collective_compute(
                "AllReduce", ALU.add, ins=[Gwo_part.ap().opt()],
                outs=[Gwo_full.ap().opt()], replica_groups=RG)

            # ========== spectral GEMM2: hencT = (new_e*P).T @ uT_r =========
            # custom kxm producer: load Pfull k-slice, scale rows by new_e
            with ExitStack() as sctx:
                s_pool = sctx.enter_context(
                    tc.tile_pool(name="s_cache", bufs=N // 512 + 1))
                s_raw = sctx.enter_context(tc.tile_pool(name="s_raw", bufs=3))
                P3 = Pfull.ap().rearrange("(po pi) f -> pi po f", pi=P)
                E1 = new_e_all.ap().rearrange("(po pi) -> pi po", pi=P)

                def s_producer(nc_, md):
                    ksub = md.k_subtiles  # 4
                    raw = s_raw.tile([P, ksub, C], BF16, tag="s_raw_t")
                    nc_.sync.dma_start(
                        raw[:], P3[:, ts(md.k_tile_idx, ksub), :])
                    esc = s_raw.tile([P, ksub], F32, tag="s_esc")
                    nc_.sync.dma_start(
                        esc[:], E1[:, ts(md.k_tile_idx, ksub)])
                    out = s_pool.tile([P, ksub, C], BF16, tag="s_cache_t")
                    for s in range(ksub):
                        nc_.scalar.activation(
                            out[:, s, :], raw[:, s, :], AF.Copy,
                            scale=esc[:, s:s + 1])
                    return out[:]

                s_shape = ShapeInfo(pdims=((P, N // P),), fdims=(C,))
                gemm(tc, None, uT_r.ap(),
                     hthfT.ap()[C:2 * C, :],
                     reducer=red_copy(), kxm_producer=s_producer,
                     kxm_shape=s_shape, kxn_bufs=8)

            # ================= attention apply =================
            # midT = (q @ Gwo).T + bo + hT + hencT   [256, R] FM
            gemm(tc, r32(Gwo_full.ap()), None, midT.ap(),
                 reducer=red_act(AF.Identity, b_o_sb),
                 consumer=(hT.ap(), hthfT.ap()[C:2 * C, :]),
                 kxm_bufs=3, kxn_producer=q_producer, kxn_shape=q_shape)

            # ================= FFN =================
            layernorm_fm(midT, fT, l2g_sb, l2b_sb, "ln2")
            # g1T = gelu(Wf1.T f + b1)               [256, R] FM
            gemm(tc, r32(w_f1.ap()), r32(fT.ap()), g1T.ap(),
                 reducer=red_act(AF.Gelu, b_f1_sb))
            # htT = Wf2.T g1 + b2 + midT             [256, R] FM -> hthfT[0:256]
            gemm(tc, r32(w_f2.ap()), r32(g1T.ap()), hthfT.ap()[0:C, :],
                 reducer=red_act(AF.Identity, b_f2_sb),
                 consumer=(midT.ap(),))

            # ================= projection heads =================
            gemm(tc, r32(w_pt1.ap()), r32(hthfT.ap()[0:C, :]), z1tT.ap(),
                 reducer=red_act(AF.Relu, b_pt1_sb))
            gemm(tc, r32(w_pt2.ap()), r32(z1tT.ap()), ztT.ap(),
                 reducer=red_act(AF.Identity, b_pt2_sb))
            gemm(tc, r32(w_pf1.ap()), r32(hthfT.ap()[C:2 * C, :]), z1fT.ap(),
                 reducer=red_act(AF.Relu, b_pf1_sb))
            gemm(tc, r32(w_pf2.ap()), r32(z1fT.ap()), zfT.ap(),
                 reducer=red_act(AF.Identity, b_pf2_sb))

            # ================= logits =================
            gemm(tc, r32(w_mo.ap()), r32(hthfT.ap()), logits_t.ap(),
                 reducer=red_act(AF.Identity, b_mo_sb))

            # ================= zf normalize + AG =================
            with ExitStack() as zctx:
                zp = zctx.enter_context(tc.tile_pool(name="zf_pool", bufs=1))
                zps = zctx.enter_context(
                    tc.tile_pool(name="zf_psum", bufs=1, space="PSUM"))
                zf_sb = zp.tile([P, R], F32R)
                nc.sync.dma_start(zf_sb[:], r32(zfT.ap()))
                zsq = zp.tile([P, R], F32R)
                nc.scalar.activation(zsq[:], zf_sb[:].bitcast(F32), AF.Square)
                ps_ss = zps.tile([1, R], F32)
                for h in range(R // 512):
                    sl = slice(h * 512, (h + 1) * 512)
                    nc.tensor.matmul(ps_ss[:, sl], ones_c[:], zsq[:, sl],
                                     start=True, stop=True)
                nrow = zp.tile([1, R], F32)
                nc.scalar.activation(nrow[:], ps_ss[:], AF.Sqrt)
                nc.vector.tensor_scalar(nrow[:], nrow[:], 1e-12, None,
                                        op0=ALU.max)
                invr = zp.tile([1, R], F32)
                nc.vector.reciprocal(invr[:], nrow[:])
                invrr = zp.tile([1, R], F32R)
                nc.scalar.copy(invrr[:], invr[:])
                ps_bc = zps.tile([P, R], F32)
                for h in range(R // 512):
                    sl = slice(h * 512, (h + 1) * 512)
                    nc.tensor.matmul(ps_bc[:, sl], ones_row[:], invrr[:, sl],
                                     start=True, stop=True)
                zfn_sb = zp.tile([P, R], F32)
                nc.vector.tensor_tensor(zfn_sb[:], zf_sb[:].bitcast(F32),
                                        ps_bc[:], op=ALU.mult)
                nc.sync.dma_start(zfnT.ap(), zfn_sb[:])
            nc.gpsimd.collective_compute(
                "AllGather", ALU.bypass, ins=[zfnT.ap().opt()],
                outs=[zfn_all.ap().opt()], replica_groups=RG)

            # ================= contrastive loss block =================
            with ExitStack() as cctx:
                cp = cctx.enter_context(tc.tile_pool(name="cl_pool", bufs=1))
                cps = cctx.enter_context(
                    tc.tile_pool(name="cl_psum", bufs=1, space="PSUM"))
                eps_pool = cctx.enter_context(
                    tc.tile_pool(name="cl_epsum", bufs=2, space="PSUM"))
                et_pool = cctx.enter_context(tc.tile_pool(name="cl_et", bufs=3))

                T8 = R // P  # 8 node tiles
                NCH = N // 512  # 16 column chunks

                # zt (unnormalized) and its row scales 1/(TEMP*||zt_i||)
                zt_sb = cp.tile([P, R], F32R)
                nc.sync.dma_start(zt_sb[:], r32(ztT.ap()))
                ztsq = cp.tile([P, R], F32)
                nc.scalar.activation(ztsq[:], zt_sb[:].bitcast(F32), AF.Square)
                ps_rs = cps.tile([P, T8], F32, name="ps_rs")
                for t in range(T8):
                    nc.tensor.matmul(ps_rs[:, t:t + 1],
                                     ztsq[:, t * P:(t + 1) * P], ones_f[:],
                                     start=True, stop=True)
                rowscale = cp.tile([P, T8], F32)
                nc.scalar.activation(rowscale[:], ps_rs[:], AF.Sqrt)
                nc.vector.tensor_scalar(rowscale[:], rowscale[:], 1e-12, None,
                                        op0=ALU.max)
                nc.vector.reciprocal(rowscale[:], rowscale[:])
                nc.vector.tensor_scalar(rowscale[:], rowscale[:], 1.0 / TEMP,
                                        None, op0=ALU.mult)

                # diag_i = rowscale_i * (zt_i . zfn_i)
                prodf = cp.tile([P, R], F32)
                zfn_loc = cp.tile([P, R], F32)
                nc.sync.dma_start(zfn_loc[:], zfnT.ap())
                nc.vector.tensor_tensor(prodf[:], zt_sb[:].bitcast(F32),
                                        zfn_loc[:], op=ALU.mult)
                ps_d = cps.tile([P, T8], F32, name="ps_d")
                for t in range(T8):
                    nc.tensor.matmul(ps_d[:, t:t + 1],
                                     prodf[:, t * P:(t + 1) * P], ones_f[:],
                                     start=True, stop=True)
                diag = cp.tile([P, T8], F32)
                nc.vector.tensor_tensor(diag[:], ps_d[:], rowscale[:],
                                        op=ALU.mult)

                # all-gathered zfn as [128, 8192]
                zfn_full = cp.tile([P, N], F32R)
                nc.sync.dma_start(
                    zfn_full[:].rearrange("p (c j) -> p c j", c=NCORES),
                    r32(zfn_all.ap().rearrange("(c f) j -> f c j", f=P)))

                stats_sb = cp.tile([1, STATS_LEN], F32)
                rsp = cp.tile([P, T8, NCH], F32)   # rowsum partials
                for c in range(NCH):
                    ps_cs = cps.tile([1, 512], F32, name="ps_cs")
                    for t in range(T8):
                        ps_l = eps_pool.tile([P, 512], F32, name="ps_l")
                        nc.tensor.matmul(
                            ps_l[:], zt_sb[:, t * P:(t + 1) * P],
                            zfn_full[:, c * 512:(c + 1) * 512],
                            start=True, stop=True)
                        e_t = et_pool.tile([P, 512], F32R, name="e_t")
                        nc.scalar.activation(
                            e_t[:], ps_l[:], AF.Exp,
                            scale=rowscale[:, t:t + 1],
                            accum_out=rsp[:, t, c:c + 1])
                        nc.tensor.matmul(ps_cs[:], ones_c[:], e_t[:],
                                         start=(t == 0), stop=(t == T8 - 1))
                    nc.scalar.copy(stats_sb[0:1, c * 512:(c + 1) * 512],
                                   ps_cs[:])

                # row logsumexp and loss partials
                rowsum = cp.tile([P, T8], F32)
                nc.vector.reduce_sum(rowsum[:], rsp[:],
                                     axis=mybir.AxisListType.X)
                rowlse = cp.tile([P, T8], F32)
                nc.scalar.activation(rowlse[:], rowsum[:], AF.Ln)
                rmd = cp.tile([P, T8], F32)
                nc.vector.tensor_tensor(rmd[:], rowlse[:], diag[:],
                                        op=ALU.subtract)
                red1 = cp.tile([P, 1], F32)
                nc.vector.reduce_sum(red1[:], rmd[:],
                                     axis=mybir.AxisListType.X)
                redd = cp.tile([P, 1], F32)
                nc.vector.reduce_sum(redd[:], diag[:],
                                     axis=mybir.AxisListType.X)
                ps_sc = cps.tile([1, 2], F32, name="ps_sc")
                nc.tensor.matmul(ps_sc[:, 0:1], red1[:], ones_f[:],
                                 start=True, stop=True)
                nc.tensor.matmul(ps_sc[:, 1:2], redd[:], ones_f[:],
                                 start=True, stop=True)
                nc.scalar.copy(stats_sb[0:1, N:N + 2], ps_sc[:])
                nc.sync.dma_start(
                    stats_in.ap().rearrange("(o a) -> o a", o=1),
                    stats_sb[:])

            nc.gpsimd.collective_compute(
                "AllReduce", ALU.add, ins=[stats_in.ap().opt()],
                outs=[stats_out.ap().opt()], replica_groups=RG)

            # ================= final scalar loss =================
            with ExitStack() as fctx:
                fp = fctx.enter_context(tc.tile_pool(name="fin_pool", bufs=1))
                fps = fctx.enter_context(
                    tc.tile_pool(name="fin_psum", bufs=1, space="PSUM"))
                cs2 = fp.tile([P, N // P], F32)
                nc.sync.dma_start(
                    cs2[:],
                    stats_out.ap()[ds(0, N)].rearrange("(a p) -> p a", p=P))
                nc.scalar.activation(cs2[:], cs2[:], AF.Ln)
                lsum = fp.tile([P, 1], F32)
                nc.vector.reduce_sum(lsum[:], cs2[:],
                                     axis=mybir.AxisListType.X)
                ps_f = fps.tile([1, 1], F32)
                nc.tensor.matmul(ps_f[:], lsum[:], ones_f[:],
                                 start=True, stop=True)
                lcs = fp.tile([1, 1], F32)
                nc.scalar.copy(lcs[:], ps_f[:])
                tail = fp.tile([1, 2], F32)
                nc.sync.dma_start(tail[:], stats_out.ap()[ds(N, 2)]
                                  .rearrange("(o a) -> o a", o=1))
                # cl = 0.5/N * (rowpart + (sum log colsum - diagsum))
                nc.vector.tensor_tensor(lcs[:], lcs[:], tail[:, 1:2],
                                        op=ALU.subtract)
                nc.vector.tensor_tensor(lcs[:], lcs[:], tail[:, 0:1],
                                        op=ALU.add)
                clv = fp.tile([1, 1], F32)
                nc.vector.tensor_scalar(clv[:], lcs[:], 0.5 / N, None,
                                        op0=ALU.mult)
                nc.sync.dma_start(cl_out.ap(), clv[:])

    nc.compile()
    return nc


# ----------------------------------------------------------------------------
# Host side
# ----------------------------------------------------------------------------

_CACHE = {}


def _get_program():
    if "nc" not in _CACHE:
        _CACHE["nc"] = build_program()
    return _CACHE["nc"]


def _softplus(x):
    return np.logaddexp(0.0, x)


def _make_in_maps(inputs):
    f = lambda k: np.asarray(inputs[k], np.float32)
    e, u, x = f("e"), f("u"), f("x")

    # eig-encoder host constants (parameter-only folding)
    deltas = _softplus(f("freq_deltas")) + 0.25
    freqs = (50.0 * np.tanh((np.cumsum(deltas) + f("freq_bias")) / 50.0)
             ).astype(np.float32)                       # [16]
    ns = np.sqrt(np.float32(NF))
    rw, rb, aw = f("read_w"), f("read_b"), f("alpha_w")
    wsin = (aw[:, None] * rw[:, 1:1 + NF] / ns).astype(np.float32)   # [10,16]
    wcos = (aw[:, None] * rw[:, 1 + NF:1 + 2 * NF] / ns).astype(np.float32)
    c0 = np.float32(np.sum(aw * (rw[:, 0] + rb)))
    enc_row = np.concatenate([freqs, wsin.ravel(), wcos.ravel(), [c0]]
                             ).astype(np.float32)       # [337]
    enc_c = np.broadcast_to(enc_row, (P, enc_row.size)).copy()

    def pcol(b):  # [L] -> [L//128, 128, 1]
        return np.ascontiguousarray(b.reshape(-1, P)[:, :, None])

    def rep(b, w):  # [w] -> [128, w]
        return np.broadcast_to(b, (P, w)).copy()

    common = {
        "w_fe1": f("fe_w1"), "w_fe2": f("fe_w2"),
        "w_q": f("wq"), "w_k": f("wk"), "w_v": f("wv"), "w_o": f("wo"),
        "w_f1": f("ffn_w1"), "w_f2": f("ffn_w2"),
        "w_pt1": f("pt_w1"), "w_pt2": f("pt_w2"),
        "w_pf1": f("pf_w1"), "w_pf2": f("pf_w2"),
        "w_mo": f("mo_w"),
        "b_fe1": pcol(f("fe_b1")), "b_fe2": pcol(f("fe_b2")),
        "b_fe2_rep": rep(f("fe_b2"), C),
        "b_q": pcol(f("bq")), "b_k_rep": rep(f("bk"), H),
        "b_v_rep": rep(f("bv"), H),
        "b_o": pcol(f("bo")),
        "b_f1": pcol(f("ffn_b1")), "b_f2": pcol(f("ffn_b2")),
        "b_pt1": pcol(f("pt_b1")), "b_pt2": pcol(f("pt_b2")),
        "b_pf1": pcol(f("pf_b1")), "b_pf2": pcol(f("pf_b2")),
        "b_mo": pcol(f("mo_b")),
        "l1g": pcol(f("ln1_g")), "l1b": pcol(f("ln1_b")),
        "l2g": pcol(f("ln2_g")), "l2b": pcol(f("ln2_b")),
        "enc_c": enc_c,
        "ones_d": np.ones((P, P), np.float32),
    }

    in_maps = []
    for r in range(NCORES):
        rows = slice(r * R, (r + 1) * R)
        u_slice = u[rows, :]
        m = dict(common)
        m["xT"] = np.ascontiguousarray(x[rows, :].T)
        m["u_r"] = np.ascontiguousarray(u_slice).astype(ml_dtypes.bfloat16)
        m["uT_r"] = np.ascontiguousarray(u_slice.T).astype(ml_dtypes.bfloat16)
        m["ep"] = np.ascontiguousarray(e[rows].reshape(R // P, P).T)
        in_maps.append(m)
    return in_maps


def kernel(trace=False, **inputs):
    nc = _get_program()
    in_maps = _make_in_maps(inputs)
    res = run_bass_kernel_spmd(nc, in_maps, list(range(NCORES)), trace=trace)
    logits = np.concatenate(
        [res.results[r]["logits_t"].T for r in range(NCORES)], axis=0)
    cl = np.float32(res.results[0]["cl_out"][0, 0])
    if trace:
        _CACHE["exec_time_ns"] = res.exec_time_ns
    return logits, cl
